# revision 9
# baseline (speedup 1.0000x reference)
"""TRN2 Bass kernel for nn_ONOBlock (linear attention + MLPs + covariance whitening).

Sharding: data-parallel over batch, 1 batch element per core (B=8, n_cores=8).
Two launches with a host boundary for the [64,64] covariance all-reduce + Cholesky:
  fx_out = X_ @ (L^-T diag(softplus(mu)) L^-1) @ (X_^T fx)
so the per-token whitening matmul disappears and only cov crosses cores.

All heavy matmuls run as float32r (round-to-nearest-11-bit-mantissa, 1 cy/row,
measured 1.5e-4 rel err). LN gains fold into the following weights on the host;
zero biases are skipped at build time (rank-1 ones-matmul fallback if nonzero).
"""
import contextlib
import numpy as np

import concourse.bass as bass
import concourse.bacc as bacc
import concourse.tile as tile
from concourse import mybir
from concourse.bass_utils import run_bass_kernel_spmd
from concourse.masks import make_identity

F32 = mybir.dt.float32
F32R = mybir.dt.float32r
AF = mybir.ActivationFunctionType
ALU = mybir.AluOpType
AX = mybir.AxisListType

B, N, D, H, PSI = 8, 7225, 256, 8, 64
DH = D // H
DF = 4 * D
EPS = 1e-5
NP_ = 7232            # padded sequence: 56*128 + 64
NCH1 = 57             # pass-1 chunks (56 of 128 + 1 of 64)
NCH2 = 15             # pass-2 chunks (14 of 512 + 1 of 64)
CORES = list(range(8))


def _bcast(ap, parts):
    """Free-dim broadcast helper: [p, g] -> [p, g, parts] with 0-stride."""
    return bass.AP(tensor=ap.tensor, offset=ap.offset,
                   ap=[ap.ap[0], ap.ap[1], [0, parts]])


def _ln_ops(nc, pool, x_ap, w, h_out, eps_t):
    """LayerNorm (no gain/bias: folded into weights): h_out[0:w] = normalize(x_ap[0:w])."""
    stats = pool.tile([128, 6], F32, tag="ln_stats")
    nc.vector.bn_stats(out=stats[0:w], in_=x_ap)
    mv = pool.tile([128, 2], F32, tag="ln_mv")
    nc.vector.bn_aggr(out=mv[0:w], in_=stats[0:w])
    rstd = pool.tile([128, 1], F32, tag="ln_rstd")
    nc.scalar.activation(rstd[0:w], mv[0:w, 1:2], AF.Sqrt, bias=eps_t[0:w])
    nc.vector.reciprocal(rstd[0:w], rstd[0:w])
    nc.vector.tensor_scalar(out=h_out[0:w], in0=x_ap, scalar1=mv[0:w, 0:1],
                            scalar2=rstd[0:w], op0=ALU.subtract, op1=ALU.mult)


def _transpose_128(nc, ptr_pool, ident_r, src_ap, w, dst_ap, copy_eng):
    """PE-transpose src_ap [w, 128] -> dst_ap [128, w] via psum.

    ident_r must match src dtype (F32R ident for F32R src, F32 for F32 src)."""
    dt_ = src_ap.dtype
    pt = ptr_pool.tile([128, 128], dt_, tag="tr", name="pt")
    nc.tensor.transpose(pt[:, 0:w], src_ap, ident_r[0:w, 0:w])
    copy_eng(dst_ap, pt[:, 0:w])


def build_launch1(flags):
    nc = bacc.Bacc(None)
    # ---- I/O ----
    x_d = nc.dram_tensor("x", [NP_, D], F32, kind="ExternalInput")
    fx_d = nc.dram_tensor("fx", [NP_, D], F32R, kind="ExternalInput")
    ones_d = nc.dram_tensor("onesp", [NP_, 8], F32R, kind="ExternalInput")
    wqkv_d = nc.dram_tensor("wqkv", [D, 3 * D], F32R, kind="ExternalInput")
    wo_d = nc.dram_tensor("wo", [D, D], F32R, kind="ExternalInput")
    w1_d = nc.dram_tensor("w1", [D, DF], F32R, kind="ExternalInput")
    w2_d = nc.dram_tensor("w2", [DF, D], F32R, kind="ExternalInput")
    p1_d = nc.dram_tensor("p1", [D, D], F32R, kind="ExternalInput")
    p2_d = nc.dram_tensor("p2", [D, PSI], F32R, kind="ExternalInput")
    cmask_d = nc.dram_tensor("cmask", [D, D], F32, kind="ExternalInput")
    ib1_d = nc.dram_tensor("ib1", [DF], F32, kind="ExternalInput")
    ip1_d = nc.dram_tensor("ip1", [D], F32, kind="ExternalInput")
    ipb2_d = nc.dram_tensor("ipb2", [PSI], F32, kind="ExternalInput")
    if flags["bqkv"]:
        bqkv_d = nc.dram_tensor("bqkv", [1, 3 * D], F32R, kind="ExternalInput")
    if flags["bo"]:
        bo_d = nc.dram_tensor("bo", [1, D], F32R, kind="ExternalInput")
    if flags["b2"]:
        b2_d = nc.dram_tensor("b2", [1, D], F32R, kind="ExternalInput")

    x2o_d = nc.dram_tensor("x2o", [NP_, D], F32, kind="ExternalOutput")
    xt_d = nc.dram_tensor("xt", [PSI, NP_], F32, kind="ExternalOutput")
    cov_d = nc.dram_tensor("cov", [PSI, PSI], F32, kind="ExternalOutput")
    c2p_d = nc.dram_tensor("c2p", [PSI, D], F32, kind="ExternalOutput")

    with tile.TileContext(nc) as tc, contextlib.ExitStack() as top:
        wp = top.enter_context(tc.tile_pool(name="wp", bufs=1))
        # ---- resident weights/constants ----
        wqkv = wp.tile([128, 2, 3 * D], F32R)
        nc.sync.dma_start(out=wqkv, in_=wqkv_d.rearrange("(c p) e -> p c e", p=128))
        wo = wp.tile([128, 2, D], F32R)
        nc.sync.dma_start(out=wo, in_=wo_d.rearrange("(c p) e -> p c e", p=128))
        w1 = wp.tile([128, 2, DF], F32R)
        nc.sync.dma_start(out=w1, in_=w1_d.rearrange("(c p) e -> p c e", p=128))
        w2 = wp.tile([128, 8, D], F32R)
        nc.sync.dma_start(out=w2, in_=w2_d.rearrange("(c p) e -> p c e", p=128))
        p1 = wp.tile([128, 2, D], F32R)
        nc.sync.dma_start(out=p1, in_=p1_d.rearrange("(c p) e -> p c e", p=128))
        p2 = wp.tile([128, 2, PSI], F32R)
        nc.sync.dma_start(out=p2, in_=p2_d.rearrange("(c p) e -> p c e", p=128))
        cmask = wp.tile([128, 2, D], F32)
        nc.sync.dma_start(out=cmask, in_=cmask_d.rearrange("(c p) e -> p c e", p=128))
        ib1 = wp.tile([128, 8], F32)
        nc.sync.dma_start(out=ib1, in_=ib1_d.rearrange("(a p) -> p a", p=128))
        ip1 = wp.tile([128, 2], F32)
        nc.sync.dma_start(out=ip1, in_=ip1_d.rearrange("(a p) -> p a", p=128))
        ipb2 = wp.tile([64, 1], F32)
        nc.sync.dma_start(out=ipb2, in_=ipb2_d.rearrange("(p a) -> p a", a=1))
        if flags["bqkv"]:
            bqkv = wp.tile([1, 3 * D], F32R)
            nc.sync.dma_start(out=bqkv, in_=bqkv_d[:])
        if flags["bo"]:
            bo = wp.tile([1, D], F32R)
            nc.sync.dma_start(out=bo, in_=bo_d[:])
        if flags["b2"]:
            b2 = wp.tile([1, D], F32R)
            nc.sync.dma_start(out=b2, in_=b2_d[:])

        eps_t = wp.tile([128, 1], F32)
        nc.vector.memset(eps_t, EPS)
        ident = wp.tile([128, 128], F32)
        make_identity(nc, ident)
        ident_r = wp.tile([128, 128], F32R)
        nc.vector.tensor_copy(ident_r, ident)
        ones_f = wp.tile([128, 16], F32)
        nc.vector.memset(ones_f, 1.0)
        ones_col = wp.tile([128, 1], F32R)
        nc.vector.tensor_copy(ones_col, ones_f[:, 0:1])
        zero_f = wp.tile([128, 16], F32)
        nc.vector.memset(zero_f, 0.0)

        qT = wp.tile([128, 2, NP_], F32R)      # q softmax'd, transposed, resident
        C_sb = wp.tile([128, 2, D], F32R)      # masked/scaled context matrix

        # ================= PASS 1 =================
        with contextlib.ExitStack() as s1:
            sb = s1.enter_context(tc.tile_pool(name="p1sb", bufs=3))
            pctx = s1.enter_context(tc.tile_pool(name="pctx", bufs=1, space="PSUM"))
            pqkv = s1.enter_context(tc.tile_pool(name="pqkv", bufs=3, space="PSUM"))
            ptr = s1.enter_context(tc.tile_pool(name="ptr", bufs=2, space="PSUM"))

            ctx_ps = [pctx.tile([128, 264], F32, tag=f"ctx{dc}", name=f"ctx_ps{dc}")
                      for dc in range(2)]

            for c in range(NCH1):
                t0 = c * 128
                w = 128 if c < NCH1 - 1 else 64

                x_sb = sb.tile([128, D], F32, tag="x_in")
                nc.sync.dma_start(out=x_sb[0:w], in_=x_d[t0:t0 + w, :])
                h0 = sb.tile([128, D], F32R, tag="h0")
                _ln_ops(nc, sb, x_sb[0:w], w, h0, eps_t)

                h0T = sb.tile([128, 2, 128], F32R, tag="h0T")
                for dc in range(2):
                    _transpose_128(nc, ptr, ident_r, h0[0:w, dc * 128:(dc + 1) * 128],
                                   w, h0T[:, dc, 0:w],
                                   lambda d_, s_: nc.scalar.activation(d_, s_, AF.Copy))

                # QKV (token layout), all three into rotating psum tiles
                ps_q = pqkv.tile([128, D], F32, tag="qkv")
                ps_k = pqkv.tile([128, D], F32, tag="qkv")
                ps_v = pqkv.tile([128, D], F32, tag="qkv")
                for i, ps in enumerate([ps_q, ps_k, ps_v]):
                    for dc in range(2):
                        nc.tensor.matmul(ps[0:w], h0T[:, dc, 0:w],
                                         wqkv[:, dc, i * D:(i + 1) * D],
                                         start=(dc == 0), stop=(dc == 1 and not flags["bqkv"]))
                    if flags["bqkv"]:
                        nc.tensor.matmul(ps[0:w], ones_col[0:1, 0:1].broadcast_to([1, w]),
                                         bqkv[:, i * D:(i + 1) * D], start=False, stop=True)

                # q: feature softmax per head (unnormalized exp, then bcast divide)
                eq = sb.tile([128, D], F32, tag="eq")
                nc.scalar.activation(eq[0:w], ps_q[0:w], AF.Exp)
                qs = sb.tile([128, 8], F32, tag="qs")
                nc.vector.reduce_sum(out=qs[0:w], in_=eq[0:w].rearrange("p (g s) -> p g s", g=8), axis=AX.X)
                nc.vector.reciprocal(qs[0:w], qs[0:w])
                q_sm = sb.tile([128, D], F32R, tag="q_sm")
                nc.vector.tensor_tensor(out=q_sm[0:w].rearrange("p (g s) -> p g s", g=8),
                                        in0=eq[0:w].rearrange("p (g s) -> p g s", g=8),
                                        in1=_bcast(qs[0:w], 32), op=ALU.mult)
                for dc in range(2):
                    _transpose_128(nc, ptr, ident_r, q_sm[0:w, dc * 128:(dc + 1) * 128],
                                   w, qT[:, dc, t0:t0 + w],
                                   lambda d_, s_: nc.scalar.activation(d_, s_, AF.Copy))

                # k: exp only; V with ones column; Z folded into ctx matmul
                ek = sb.tile([128, D], F32R, tag="ek")
                nc.scalar.activation(ek[0:w], ps_k[0:w], AF.Exp)
                v_sb = sb.tile([128, 264], F32R, tag="v_sb")
                nc.scalar.activation(v_sb[0:w, 0:D], ps_v[0:w], AF.Copy)
                nc.sync.dma_start(out=v_sb[0:w, D:264], in_=ones_d[t0:t0 + w, :])
                for dc in range(2):
                    nc.tensor.matmul(ctx_ps[dc], ek[0:w, dc * 128:(dc + 1) * 128],
                                     v_sb[0:w, :], start=(c == 0), stop=(c == NCH1 - 1))

            for dc in range(2):
                nc.vector.tensor_copy(qT[:, dc, N:NP_], zero_f[:, 0:NP_ - N])

            # ---- build C = blockdiag_mask * DH^-0.5 * diag(1/Z) @ ctx ----
            for dc in range(2):
                zr = sb.tile([128, 1], F32, tag="zr")
                nc.vector.reciprocal(zr, ctx_ps[dc][:, 256:257])
                ct = sb.tile([128, D], F32, tag="ct")
                nc.vector.tensor_scalar(out=ct, in0=ctx_ps[dc][:, 0:D], scalar1=zr,
                                        scalar2=None, op0=ALU.mult)
                nc.vector.tensor_tensor(out=C_sb[:, dc, :], in0=ct, in1=cmask[:, dc, :], op=ALU.mult)

        # ================= PASS 2 =================
        with contextlib.ExitStack() as s2:
            sb = s2.enter_context(tc.tile_pool(name="p2sb", bufs=2))
            sb3 = s2.enter_context(tc.tile_pool(name="p2sb3", bufs=3))
            pcc = s2.enter_context(tc.tile_pool(name="pcc", bufs=1, space="PSUM"))
            pbig = s2.enter_context(tc.tile_pool(name="pbig", bufs=2, space="PSUM"))
            px2 = s2.enter_context(tc.tile_pool(name="px2", bufs=2, space="PSUM"))
            ptr = s2.enter_context(tc.tile_pool(name="ptr2", bufs=2, space="PSUM"))

            cc_ps = pcc.tile([64, 320], F32)

            for C in range(NCH2):
                T0 = C * 512
                T = 512 if C < NCH2 - 1 else 64
                nsub = T // 128 if C < NCH2 - 1 else 1
                sw = 128 if C < NCH2 - 1 else 64

                # attention application: attnT[e, tok] = C^T blocks @ qT
                attnT = sb.tile([128, 2, 512], F32R, tag="attnT")
                for ec in range(2):
                    aps = pbig.tile([128, 512], F32, tag="big")
                    for dc in range(2):
                        nc.tensor.matmul(aps[:, 0:T], C_sb[:, dc, ec * 128:(ec + 1) * 128],
                                         qT[:, dc, T0:T0 + T], start=(dc == 0), stop=(dc == 1))
                    nc.vector.tensor_copy(attnT[:, ec, 0:T], aps[:, 0:T])

                x1_sb = sb.tile([128, 4, D], F32, tag="x1")
                h2T = sb.tile([128, 2, 512], F32R, tag="h2T")
                for s in range(nsub):
                    t0 = T0 + s * 128
                    v = sw if C < NCH2 - 1 else N - t0  # 57 in last chunk
                    xps = pbig.tile([128, 512], F32, tag="big")
                    for ec in range(2):
                        nc.tensor.matmul(xps[0:sw, 0:D], attnT[:, ec, s * 128:s * 128 + sw],
                                         wo[:, ec, :],
                                         start=(ec == 0), stop=(ec == 1 and not flags["bo"]))
                    if flags["bo"]:
                        nc.tensor.matmul(xps[0:sw, 0:D], ones_col[0:1, 0:1].broadcast_to([1, sw]),
                                         bo[:], start=False, stop=True)
                    x_in = sb3.tile([128, D], F32, tag="x_in2")
                    nc.sync.dma_start(out=x_in[0:sw], in_=x_d[t0:t0 + sw, :])
                    nc.vector.tensor_tensor(out=x1_sb[0:sw, s, :], in0=xps[0:sw, 0:D],
                                            in1=x_in[0:sw], op=ALU.add)
                    h2 = sb3.tile([128, D], F32R, tag="h2")
                    _ln_ops(nc, sb3, x1_sb[0:sw, s, :], sw, h2, eps_t)
                    for dc in range(2):
                        _transpose_128(nc, ptr, ident_r, h2[0:sw, dc * 128:(dc + 1) * 128],
                                       sw, h2T[:, dc, s * 128:s * 128 + sw],
                                       lambda d_, s_: nc.vector.tensor_copy(d_, s_))

                # MLP: u[f, tok] = gelu(W1^T h2T + ib1), all 8 f-slices resident
                uT = sb.tile([128, 8, 512], F32R, tag="uT")
                for fs in range(8):
                    ups = pbig.tile([128, 512], F32, tag="big")
                    for dc in range(2):
                        nc.tensor.matmul(ups[:, 0:T], w1[:, dc, fs * 128:(fs + 1) * 128],
                                         h2T[:, dc, 0:T], start=(dc == 0), stop=(dc == 1))
                    nc.scalar.activation(uT[:, fs, 0:T], ups[:, 0:T], AF.Gelu,
                                         bias=ib1[:, fs:fs + 1])

                x2T = sb.tile([128, 2, 512], F32R, tag="x2T")
                for s in range(nsub):
                    t0 = T0 + s * 128
                    x2ps = px2.tile([128, D], F32, tag="x2ps")
                    for fs in range(8):
                        nc.tensor.matmul(x2ps[0:sw], uT[:, fs, s * 128:s * 128 + sw],
                                         w2[:, fs, :],
                                         start=(fs == 0), stop=(fs == 7 and not flags["b2"]))
                    if flags["b2"]:
                        nc.tensor.matmul(x2ps[0:sw], ones_col[0:1, 0:1].broadcast_to([1, sw]),
                                         b2[:], start=False, stop=True)
                    x2_sb = sb3.tile([128, D], F32, tag="x2_sb")
                    nc.vector.tensor_tensor(out=x2_sb[0:sw], in0=x2ps[0:sw],
                                            in1=x1_sb[0:sw, s, :], op=ALU.add)
                    nc.sync.dma_start(out=x2o_d[t0:t0 + sw, :], in_=x2_sb[0:sw])
                    for dc in range(2):
                        _transpose_128(nc, ptr, ident, x2_sb[0:sw, dc * 128:(dc + 1) * 128],
                                       sw, x2T[:, dc, s * 128:s * 128 + sw],
                                       lambda d_, s_: nc.vector.tensor_copy(d_, s_))

                # proj: pT = gelu(P1^T x2T + ip1); xT = P2^T pT (+ipb2)
                pT = sb.tile([128, 2, 512], F32R, tag="pT")
                for pc in range(2):
                    pps = pbig.tile([128, 512], F32, tag="big")
                    for dc in range(2):
                        nc.tensor.matmul(pps[:, 0:T], p1[:, dc, pc * 128:(pc + 1) * 128],
                                         x2T[:, dc, 0:T], start=(dc == 0), stop=(dc == 1))
                    nc.scalar.activation(pT[:, pc, 0:T], pps[:, 0:T], AF.Gelu,
                                         bias=ip1[:, pc:pc + 1])
                xtps = pbig.tile([128, 512], F32, tag="big")
                for pc in range(2):
                    nc.tensor.matmul(xtps[0:64, 0:T], p2[:, pc, :], pT[:, pc, 0:T],
                                     start=(pc == 0), stop=(pc == 1))
                xT_sb = sb.tile([64, 512], F32R, tag="xT_sb")
                nc.scalar.activation(xT_sb[:, 0:T], xtps[0:64, 0:T], AF.Identity,
                                     bias=ipb2[:, 0:1])
                nc.sync.dma_start(out=xt_d[:, T0:T0 + T], in_=xT_sb[:, 0:T].bitcast(F32))

                # x_ token-layout + fx concat -> cov/ctx2' accumulation
                for s in range(nsub):
                    t0 = T0 + s * 128
                    vv = min(sw, N - t0)
                    xc = sb3.tile([128, 320], F32R, tag="xc")
                    xtr = ptr.tile([128, 128], F32R, tag="tr")
                    nc.tensor.transpose(xtr[0:sw, 0:64], xT_sb[:, s * 128:s * 128 + sw],
                                        ident_r[0:64, 0:64])
                    if vv < sw and flags.get("anybias"):
                        # nonzero-bias pads flow nonzero x_; mask them out of cov
                        nc.vector.tensor_copy(xc[0:sw, :],
                                              _bcast(zero_f[0:sw, 0:1], 320).rearrange("p a b -> p (a b)"))
                        nc.vector.tensor_copy(xc[0:vv, 0:64], xtr[0:vv, 0:64])
                    else:
                        nc.vector.tensor_copy(xc[0:sw, 0:64], xtr[0:sw, 0:64])
                    nc.sync.dma_start(out=xc[0:sw, 64:320], in_=fx_d[t0:t0 + sw, :])
                    nc.tensor.matmul(cc_ps, xc[0:sw, 0:64], xc[0:sw, :],
                                     start=(C == 0 and s == 0),
                                     stop=(C == NCH2 - 1 and s == nsub - 1))

            cc_sb = sb.tile([64, 320], F32, tag="cc_sb")
            nc.vector.tensor_copy(cc_sb, cc_ps)
            nc.sync.dma_start(out=cov_d[:], in_=cc_sb[:, 0:64])
            nc.sync.dma_start(out=c2p_d[:], in_=cc_sb[:, 64:320])

    nc.finalize()
    return nc


def build_launch2(flags):
    nc = bacc.Bacc(None)
    xt_d = nc.dram_tensor("xt", [PSI, NP_], F32R, kind="ExternalInput")
    c2pp_d = nc.dram_tensor("c2pp", [PSI, D], F32R, kind="ExternalInput")
    m1_d = nc.dram_tensor("m1", [D, DF], F32R, kind="ExternalInput")
    m2_d = nc.dram_tensor("m2", [DF, D], F32R, kind="ExternalInput")
    ib2_d = nc.dram_tensor("ib2", [DF], F32, kind="ExternalInput")
    if flags["mb2"]:
        mb2_d = nc.dram_tensor("mb2", [1, D], F32R, kind="ExternalInput")
    fxo_d = nc.dram_tensor("fxo", [NP_, D], F32, kind="ExternalOutput")

    with tile.TileContext(nc) as tc, contextlib.ExitStack() as top:
        wp = top.enter_context(tc.tile_pool(name="wp", bufs=1))
        xt_all = wp.tile([64, NP_], F32R)
        nc.sync.dma_start(out=xt_all, in_=xt_d[:])
        c2pp = wp.tile([64, D], F32R)
        nc.sync.dma_start(out=c2pp, in_=c2pp_d[:])
        m1 = wp.tile([128, 2, DF], F32R)
        nc.sync.dma_start(out=m1, in_=m1_d.rearrange("(c p) e -> p c e", p=128))
        m2 = wp.tile([128, 8, D], F32R)
        nc.sync.dma_start(out=m2, in_=m2_d.rearrange("(c p) e -> p c e", p=128))
        ib2 = wp.tile([128, 8], F32)
        nc.sync.dma_start(out=ib2, in_=ib2_d.rearrange("(a p) -> p a", p=128))
        if flags["mb2"]:
            mb2 = wp.tile([1, D], F32R)
            nc.sync.dma_start(out=mb2, in_=mb2_d[:])
            ones_f = wp.tile([128, 1], F32)
            nc.vector.memset(ones_f, 1.0)
            ones_col = wp.tile([128, 1], F32R)
            nc.vector.tensor_copy(ones_col, ones_f)
        eps_t = wp.tile([128, 1], F32)
        nc.vector.memset(eps_t, EPS)
        ident = wp.tile([128, 128], F32)
        make_identity(nc, ident)
        ident_r = wp.tile([128, 128], F32R)
        nc.vector.tensor_copy(ident_r, ident)

        with contextlib.ExitStack() as s1:
            sb = s1.enter_context(tc.tile_pool(name="sb", bufs=2))
            sb3 = s1.enter_context(tc.tile_pool(name="sb3", bufs=3))
            pbig = s1.enter_context(tc.tile_pool(name="pbig", bufs=2, space="PSUM"))
            pmid = s1.enter_context(tc.tile_pool(name="pmid", bufs=2, space="PSUM"))
            ptr = s1.enter_context(tc.tile_pool(name="ptr", bufs=2, space="PSUM"))

            for C in range(NCH2):
                T0 = C * 512
                T = 512 if C < NCH2 - 1 else 64
                nsub = T // 128 if C < NCH2 - 1 else 1
                sw = 128 if C < NCH2 - 1 else 64

                h3T = sb.tile([128, 2, 512], F32R, tag="h3T")
                for s in range(nsub):
                    t0 = T0 + s * 128
                    fps = pmid.tile([128, D], F32, tag="fxu")
                    nc.tensor.matmul(fps[0:sw], xt_all[:, t0:t0 + sw], c2pp[:],
                                     start=True, stop=True)
                    h3 = sb3.tile([128, D], F32R, tag="h3")
                    _ln_ops(nc, sb3, fps[0:sw], sw, h3, eps_t)
                    for dc in range(2):
                        _transpose_128(nc, ptr, ident_r, h3[0:sw, dc * 128:(dc + 1) * 128],
                                       sw, h3T[:, dc, s * 128:s * 128 + sw],
                                       lambda d_, s_: nc.vector.tensor_copy(d_, s_))

                uT = sb.tile([128, 8, 512], F32R, tag="uT")
                for fs in range(8):
                    ups = pbig.tile([128, 512], F32, tag="big")
                    for dc in range(2):
                        nc.tensor.matmul(ups[:, 0:T], m1[:, dc, fs * 128:(fs + 1) * 128],
                                         h3T[:, dc, 0:T], start=(dc == 0), stop=(dc == 1))
                    nc.scalar.activation(uT[:, fs, 0:T], ups[:, 0:T], AF.Gelu,
                                         bias=ib2[:, fs:fs + 1])

                for s in range(nsub):
                    t0 = T0 + s * 128
                    ops_ = pmid.tile([128, D], F32, tag="fout")
                    for fs in range(8):
                        nc.tensor.matmul(ops_[0:sw], uT[:, fs, s * 128:s * 128 + sw],
                                         m2[:, fs, :],
                                         start=(fs == 0), stop=(fs == 7 and not flags["mb2"]))
                    if flags["mb2"]:
                        nc.tensor.matmul(ops_[0:sw], ones_col[0:1, 0:1].broadcast_to([1, sw]),
                                         mb2[:], start=False, stop=True)
                    fo = sb3.tile([128, D], F32, tag="fo")
                    nc.vector.tensor_copy(fo[0:sw], ops_[0:sw])
                    nc.sync.dma_start(out=fxo_d[t0:t0 + sw, :], in_=fo[0:sw])

    nc.finalize()
    return nc


_NC_CACHE = {}


def _get_nc(which, flags):
    key = (which, tuple(sorted(flags.items())))
    if key not in _NC_CACHE:
        _NC_CACHE[key] = build_launch1(flags) if which == 1 else build_launch2(flags)
    return _NC_CACHE[key]


def kernel(**inputs):
    inp = {k: np.ascontiguousarray(np.asarray(v)) for k, v in inputs.items()}
    x, fx = inp["x"], inp["fx"]
    f64 = lambda k: inp[k].astype(np.float64)

    # ---- host-side weight folding (LN gains into following weights) ----
    g1, b1 = f64("ln1_g"), f64("ln1_b")
    g2, b2 = f64("ln2_g"), f64("ln2_b")
    g3, b3 = f64("ln3_g"), f64("ln3_b")
    Wq, Wk, Wv = f64("Wq"), f64("Wk"), f64("Wv")
    wqkv = np.concatenate([g1[:, None] * Wq, g1[:, None] * Wk, g1[:, None] * Wv],
                          axis=1).astype(np.float32)
    bqkv = np.concatenate([b1 @ Wq, b1 @ Wk, b1 @ Wv]).astype(np.float32)[None, :]
    w1 = (g2[:, None] * f64("mlp_W1")).astype(np.float32)
    ib1 = (b2 @ f64("mlp_W1") + f64("mlp_b1")).astype(np.float32)
    m1 = (g3[:, None] * f64("mlp2_W1")).astype(np.float32)
    ib2 = (b3 @ f64("mlp2_W1") + f64("mlp2_b1")).astype(np.float32)
    cmask = np.zeros((D, D), np.float32)
    for h in range(H):
        cmask[h * DH:(h + 1) * DH, h * DH:(h + 1) * DH] = DH ** -0.5

    flags1 = {"bqkv": bool(np.any(bqkv)), "bo": bool(np.any(inp["bo"])),
              "b2": bool(np.any(inp["mlp_b2"]))}
    flags1["anybias"] = any(flags1.values()) or bool(np.any(ib1)) or bool(np.any(inp["proj_b1"])) or bool(np.any(inp["proj_b2"]))
    xp = np.zeros((B, NP_, D), np.float32); xp[:, :N] = x
    fxp = np.zeros((B, NP_, D), np.float32); fxp[:, :N] = fx
    onesp = np.zeros((NP_, 8), np.float32); onesp[:N] = 1.0
    flags2 = {"mb2": bool(np.any(inp["mlp2_b2"]))}

    common1 = {
        "wqkv": wqkv, "wo": inp["Wo"], "w1": w1, "w2": inp["mlp_W2"],
        "p1": inp["proj_W1"], "p2": inp["proj_W2"], "cmask": cmask,
        "ib1": ib1, "ip1": inp["proj_b1"], "ipb2": inp["proj_b2"],
    }
    if flags1["bqkv"]:
        common1["bqkv"] = bqkv
    if flags1["bo"]:
        common1["bo"] = inp["bo"][None, :].astype(np.float32)
    if flags1["b2"]:
        common1["b2"] = inp["mlp_b2"][None, :].astype(np.float32)

    nc1 = _get_nc(1, flags1)
    in_maps1 = [dict(common1, x=xp[b], fx=fxp[b], onesp=onesp) for b in range(B)]
    res1 = run_bass_kernel_spmd(nc1, in_maps1, CORES).results
    res1 = [{k: np.asarray(v) for k, v in r.items()} for r in res1]

    # ---- host boundary: cov all-reduce + Cholesky + M fold ----
    cov = sum(r["cov"].astype(np.float64) for r in res1) / (B * N)
    L = np.linalg.cholesky(cov)
    Linv = np.linalg.inv(L)
    sp_mu = np.log1p(np.exp(inp["mu"].astype(np.float64)))
    M = Linv.T @ (sp_mu[:, None] * Linv)

    common2 = {"m1": m1, "m2": inp["mlp2_W2"], "ib2": ib2}
    if flags2["mb2"]:
        common2["mb2"] = inp["mlp2_b2"][None, :].astype(np.float32)
    nc2 = _get_nc(2, flags2)
    in_maps2 = [dict(common2, xt=res1[b]["xt"],
                     c2pp=(M @ res1[b]["c2p"].astype(np.float64)).astype(np.float32))
                for b in range(B)]
    res2 = run_bass_kernel_spmd(nc2, in_maps2, CORES).results
    res2 = [{k: np.asarray(v) for k, v in r.items()} for r in res2]

    x_out = np.stack([res1[b]["x2o"][:N] for b in range(B)]).astype(np.float32)
    fx_out = np.stack([res2[b]["fxo"][:N] for b in range(B)]).astype(np.float32)
    return x_out, fx_out
